# revision 17
# baseline (speedup 1.0000x reference)
"""DCT-based 1.25x upsample (2D DCT-II -> zero-pad spectrum -> 2D IDCT).

The reference computation is linear per (b, c) slice: out = M @ x @ M^T with
M = E960[:, :768] @ D768 (960x768). M is *centrosymmetric*
(M[959-i, 767-n] = M[i, n]), so the symmetric/antisymmetric fold halves the
matmul FLOPs:

    MP = (M[:480, :384] + M[:480, 767:383:-1]) / 2     [480, 384]
    MM = (M[:480, :384] - M[:480, 767:383:-1]) / 2
    x_pq = (row-fold p)(col-fold q)(x)                 4 tiles of [384, 384]

    P1 = x_pp MP^T   P2 = x_pm MM^T   P3 = x_mp MP^T   P4 = x_mm MM^T
    A = P1+P2  C = P1-P2  B = P3+P4  D = P3-P4         [384, 480] each
    Q1 = MP A   Q2 = MM B   Q3 = MP C   Q4 = MM D      [480, 480] each
    out quadrants = (Q1 +- Q2, Q3 +- Q4) with flips    (done on HOST)

MP is itself centrosymmetric (MM is not), giving a second-level fold on the
Q1/Q3 products: with x_pp/x_pm row-PERMUTED on the host into level-2 folded
order (rows [0:192) = r[n]+r[383-n], [192:384) = r[n]-r[383-n]), A and C come
out of stage 1 already row-folded, and

    Q1 = unfold_i([MP2p @ A[:192] ; MP2m @ A[192:]])   MP2{p,m} [240, 192]

costs 8 matmuls instead of 12 (the unfold is part of the host assembly).

Host does all O(N^2) folds/permutes/unfolds/flips (free, like the layout
striping); the device runs bf16 matmuls (1 PE cycle/row) plus the minimal
PSUM drains: stage 1 needs one ScalarE cast + two DVE PSUM ops per step
(A/C butterfly feeds stage 2 on-device), stage 2 is pure casts split between
ScalarE and DVE. GpSimd has no PSUM port and DMA no PSUM route, so those
stay off the drain path.
"""

import numpy as np
import ml_dtypes

import concourse.bass as bass  # noqa: F401  (engine types route via nc)
import concourse.mybir as mybir
import concourse.tile as tile
from concourse import bacc
from concourse.bass_utils import run_bass_kernel_spmd

# Problem shape (hardcoded per contract)
B, C, H = 16, 3, 768
OUT = 960  # H * 1.25
N_CORES = 8
SLICES = (B * C) // N_CORES  # 6 per core

P = 128
HF = H // 2     # 384: folded input length
QF = OUT // 2   # 480: folded output length
H2 = HF // 2    # 192: doubly folded (level-2 contraction)
Q2F = QF // 2   # 240: doubly folded output rows
KT = HF // P    # 3 contraction tiles of 128
NT1 = HF // P   # 3 stage-1 output-row tiles
M2 = 120
MT2 = QF // M2  # 4 stage-2 output-row tiles

DT = mybir.dt.bfloat16
BF16 = ml_dtypes.bfloat16


def _build_consts():
    """Returns (mt, mt2a, mt2b) host arrays (bf16).

    mt   [128, 2*KT*QF]: MP^T / MM^T, K striped -> (kl, p). Used as the
         stage-1 moving operand and the stage-2 lhsT for Q2/Q4.
    mt2a [128, 2*Q2F], mt2b [64, 2*Q2F]: level-2 constants MP2p^T/MP2m^T
         [192, 240] split into the K-chunks matching A's row tiling:
         f+ contracts A rows 0:192  = [kl0 (128) ; kl1[:64]  (64)]
         f- contracts A rows 192:384 = [kl1[64:] (64) ; kl2 (128)]
         mt2a[:, 0] = MP2pT[0:128]   mt2b[:, 0] = MP2pT[128:192]
         mt2b[:, 1] = MP2mT[0:64]    mt2a[:, 1] = MP2mT[64:192]
    """
    n = np.arange(H, dtype=np.float64)
    k = np.arange(H, dtype=np.float64)[:, None]
    D = 2.0 * np.cos(np.pi * (2.0 * n[None, :] + 1.0) * k / (2.0 * H))

    n2 = np.arange(OUT, dtype=np.float64)[:, None]
    k2 = np.arange(OUT, dtype=np.float64)[None, :]
    E = np.cos(np.pi * (2.0 * n2 + 1.0) * k2 / (2.0 * OUT)) / OUT
    E[:, 0] = 1.0 / (2.0 * OUT)

    M = E[:, :H] @ D  # [960, 768]
    MP = (M[:QF, :HF] + M[:QF, H - 1 : HF - 1 : -1]) / 2.0  # [480, 384]
    MM = (M[:QF, :HF] - M[:QF, H - 1 : HF - 1 : -1]) / 2.0
    mt = np.stack([MP.T, MM.T])  # [2, 384, 480]
    mt = mt.reshape(2, KT, P, QF).transpose(2, 0, 1, 3)
    mt = np.ascontiguousarray(mt.reshape(P, 2 * KT * QF)).astype(BF16)

    MP2pT = ((MP[:Q2F, :H2] + MP[:Q2F, HF - 1 : H2 - 1 : -1]) / 2.0).T  # [192, 240]
    MP2mT = ((MP[:Q2F, :H2] - MP[:Q2F, HF - 1 : H2 - 1 : -1]) / 2.0).T
    mt2a = np.stack([MP2pT[0:128], MP2mT[64:192]], axis=1)  # [128, 2, 240]
    # 64-row chunks parked at the base partition their rhs slice uses
    # (matmul requires lhsT/rhs base partitions to match):
    # [:64, 0] = MP2pT[128:192] (rhs base 0), [64:, 1] = MP2mT[0:64] (base 64)
    mt2b = np.zeros((P, 2, Q2F), dtype=np.float64)
    mt2b[:64, 0] = MP2pT[128:192]
    mt2b[64:, 1] = MP2mT[0:64]
    mt2a = np.ascontiguousarray(mt2a.reshape(P, 2 * Q2F)).astype(BF16)
    mt2b = np.ascontiguousarray(mt2b.reshape(P, 2 * Q2F)).astype(BF16)
    return mt, mt2a, mt2b


def _fold_inputs(x: np.ndarray) -> np.ndarray:
    """Host fold + lhsT striping: [B*C, 128, 4*KT*HF] bf16.

    Product order t=0..3 pairs (x_pp, MPt), (x_pm, MMt), (x_mp, MPt),
    (x_mm, MMt). x_pp and x_pm (the A/C path) are additionally row-permuted
    into level-2 folded order. lhsT layout: x_sb[p, t, kl, n] =
    x_t[n, kl*128+p].
    """
    xr = x.reshape(B * C, H, H)
    fp = xr[:, :, :HF] + xr[:, :, H - 1 : HF - 1 : -1]  # col fold
    fm = xr[:, :, :HF] - xr[:, :, H - 1 : HF - 1 : -1]
    xpp = fp[:, :HF] + fp[:, H - 1 : HF - 1 : -1]  # row fold
    xmp = fp[:, :HF] - fp[:, H - 1 : HF - 1 : -1]
    xpm = fm[:, :HF] + fm[:, H - 1 : HF - 1 : -1]
    xmm = fm[:, :HF] - fm[:, H - 1 : HF - 1 : -1]
    # level-2 row fold (permute) on the A/C path
    xpp = np.concatenate(
        [xpp[:, :H2] + xpp[:, HF - 1 : H2 - 1 : -1],
         xpp[:, :H2] - xpp[:, HF - 1 : H2 - 1 : -1]], axis=1)
    xpm = np.concatenate(
        [xpm[:, :H2] + xpm[:, HF - 1 : H2 - 1 : -1],
         xpm[:, :H2] - xpm[:, HF - 1 : H2 - 1 : -1]], axis=1)
    xs = np.stack([xpp, xpm, xmp, xmm], axis=1)  # [B*C, 4, n, m]
    xt = xs.transpose(0, 1, 3, 2)  # lhsT: [B*C, 4, m, n]
    xt = xt.reshape(B * C, 4, KT, P, HF).transpose(0, 3, 1, 2, 4)
    return np.ascontiguousarray(xt.reshape(B * C, P, 4 * KT * HF)).astype(BF16)


def _build_program():
    nc = bacc.Bacc(None, target_bir_lowering=False, debug=False)

    x_ext = nc.dram_tensor("x", [SLICES, P, 4 * KT * HF], DT, kind="ExternalInput")
    mt_ext = nc.dram_tensor("mt", [P, 2 * KT * QF], DT, kind="ExternalInput")
    mt2a_ext = nc.dram_tensor("mt2a", [P, 2 * Q2F], DT, kind="ExternalInput")
    mt2b_ext = nc.dram_tensor("mt2b", [P, 2 * Q2F], DT, kind="ExternalInput")
    out_ext = nc.dram_tensor("out", [SLICES, 4, QF, QF], DT, kind="ExternalOutput")

    with tile.TileContext(nc) as tc:
        with (
            tc.tile_pool(name="const", bufs=1) as const_pool,
            tc.tile_pool(name="xp", bufs=3) as x_pool,
            tc.tile_pool(name="rp", bufs=2) as r_pool,
            tc.tile_pool(name="op", bufs=10) as o_pool,
            tc.tile_pool(name="ps", bufs=8, space="PSUM") as psum_pool,
        ):
            x_dram = x_ext[:].rearrange("s p (t k n) -> s p t k n", t=4, k=KT)

            # Slice-0 x rides two HWDGE queues so stage-1 pair 0's half
            # lands by ~9us; constants share the scalar queue ahead of the
            # second half. Later slices prefetch on sync with plenty of slack.
            x_first = x_pool.tile([P, 4, KT, HF], DT, tag="x")
            nc.sync.dma_start(x_first[:, 0:2], x_dram[0][:, 0:2])

            mt_dram = mt_ext[:].rearrange("p (c k j) -> p c k j", c=2, k=KT)
            mt_sb = const_pool.tile([P, 2, KT, QF], DT, name="mt")
            nc.scalar.dma_start(mt_sb[:], mt_dram[:])
            mt2a_sb = const_pool.tile([P, 2, Q2F], DT, name="mt2a")
            nc.scalar.dma_start(mt2a_sb[:], mt2a_ext[:].rearrange("p (c j) -> p c j", c=2))
            mt2b_sb = const_pool.tile([P, 2, Q2F], DT, name="mt2b")
            nc.scalar.dma_start(mt2b_sb[:], mt2b_ext[:].rearrange("p (c j) -> p c j", c=2))
            nc.scalar.dma_start(x_first[:, 2:4], x_dram[0][:, 2:4])

            # PE warmup: dummy matmuls keep the tensor engine busy while the
            # first loads land, so the HAM clock gate is already at 2.4 GHz.
            warm_m = const_pool.tile([P, QF], DT, name="warm_m")
            nc.vector.memset(warm_m[:], 0.0)
            warm_ps = psum_pool.tile([P, QF], mybir.dt.float32, tag="ps", name="warm_ps")
            for _ in range(11):
                nc.tensor.matmul(warm_ps[:], warm_m[:, :P], warm_m[:], start=True, stop=True)

            for s in range(SLICES):
                if s == 0:
                    x_sb = x_first
                else:
                    x_sb = x_pool.tile([P, 4, KT, HF], DT, tag="x")
                    nc.sync.dma_start(x_sb[:], x_dram[s])

                # Stage 1: P_t = x_t @ {MP,MM}^T, combined in pairs into
                # r_sb[:, u]: u=0:A, u=1:C (level-2-row-folded), u=2:B, u=3:D,
                # K-striped for stage 2: r_sb[p, u, nt, j] = U[nt*128+p, j].
                r_sb = r_pool.tile([P, 4, KT, QF], DT, tag="r")
                for pair in range(2):
                    for nt in range(NT1):
                        ps_a = psum_pool.tile([P, QF], mybir.dt.float32, tag="ps")
                        ps_b = psum_pool.tile([P, QF], mybir.dt.float32, tag="ps")
                        for kl in range(KT):
                            nc.tensor.matmul(
                                ps_a[:],
                                x_sb[:, 2 * pair, kl, nt * P : (nt + 1) * P],
                                mt_sb[:, 0, kl, :],
                                start=(kl == 0),
                                stop=(kl == KT - 1),
                            )
                        for kl in range(KT):
                            nc.tensor.matmul(
                                ps_b[:],
                                x_sb[:, 2 * pair + 1, kl, nt * P : (nt + 1) * P],
                                mt_sb[:, 1, kl, :],
                                start=(kl == 0),
                                stop=(kl == KT - 1),
                            )
                        # Minimal drain: ScalarE casts ps_b (DVE has a single
                        # PSUM read port, dual-PSUM tensor_tensor is illegal);
                        # DVE butterflies PSUM + SBUF -> SBUF bf16.
                        pb_sb = o_pool.tile([P, QF], mybir.dt.float32, tag="o")
                        nc.scalar.copy(pb_sb[:], ps_b[:])
                        nc.vector.tensor_add(
                            out=r_sb[:, 2 * pair, nt, :], in0=ps_a[:], in1=pb_sb[:]
                        )
                        nc.vector.tensor_sub(
                            out=r_sb[:, 2 * pair + 1, nt, :], in0=ps_a[:], in1=pb_sb[:]
                        )

                # Stage 2: Q1 = MP A, Q3 = MP C via level-2 fold (8 matmuls,
                # output rows = [f+ (240) ; f- (240)], host unfolds);
                # Q2 = MM B, Q4 = MM D natural (12 matmuls). Drains are pure
                # casts, alternating ScalarE / DVE. Host does the +- butterfly.
                for qi, u, folded in ((0, 0, True), (1, 2, False), (2, 1, True), (3, 3, False)):
                    for mi in range(MT2):
                        ps = psum_pool.tile([P, QF], mybir.dt.float32, tag="ps")
                        po = ps[:M2, :]
                        if folded:
                            half, mi2b = divmod(mi, 2)
                            msl = slice(mi2b * M2, (mi2b + 1) * M2)
                            if half == 0:  # f+ : A rows 0:192
                                nc.tensor.matmul(
                                    po, mt2a_sb[:, 0, msl],
                                    r_sb[:, u, 0, :], start=True, stop=False,
                                )
                                nc.tensor.matmul(
                                    po, mt2b_sb[:64, 0, msl],
                                    r_sb[:64, u, 1, :], start=False, stop=True,
                                )
                            else:  # f- : A rows 192:384
                                nc.tensor.matmul(
                                    po, mt2b_sb[64:, 1, msl],
                                    r_sb[64:, u, 1, :], start=True, stop=False,
                                )
                                nc.tensor.matmul(
                                    po, mt2a_sb[:, 1, msl],
                                    r_sb[:, u, 2, :], start=False, stop=True,
                                )
                        else:
                            for kl in range(KT):
                                nc.tensor.matmul(
                                    po,
                                    mt_sb[:, 1, kl, mi * M2 : (mi + 1) * M2],
                                    r_sb[:, u, kl, :],
                                    start=(kl == 0),
                                    stop=(kl == KT - 1),
                                )
                        oc = o_pool.tile([M2, QF], DT, tag="o")
                        if (qi * MT2 + mi) % 2 == 0:
                            nc.scalar.copy(oc[:], po)
                            nc.scalar.dma_start(
                                out_ext[s, qi, mi * M2 : (mi + 1) * M2, :], oc[:]
                            )
                        else:
                            nc.vector.tensor_copy(oc[:], po)
                            nc.sync.dma_start(
                                out_ext[s, qi, mi * M2 : (mi + 1) * M2, :], oc[:]
                            )

    nc.compile()
    return nc


_CACHE: dict = {}


def _get_program():
    if "nc" not in _CACHE:
        _CACHE["nc"] = _build_program()
        _CACHE["consts"] = _build_consts()
    return _CACHE["nc"], _CACHE["consts"]


def kernel(x: np.ndarray, _trace: bool = False):
    assert x.shape == (B, C, H, H), x.shape
    nc, (mt, mt2a, mt2b) = _get_program()
    x = np.ascontiguousarray(x, dtype=np.float32)
    x_arr = _fold_inputs(x)
    per_core = B // N_CORES
    in_maps = [
        {"x": x_arr[i * SLICES : (i + 1) * SLICES], "mt": mt, "mt2a": mt2a,
         "mt2b": mt2b}
        for i in range(N_CORES)
    ]
    res = run_bass_kernel_spmd(nc, in_maps, list(range(N_CORES)), trace=_trace)
    out = np.empty((B, C, OUT, OUT), dtype=np.float32)
    for i in range(N_CORES):
        q = np.asarray(res.results[i]["out"]).astype(np.float32)
        q = q.reshape(per_core, C, 4, QF, QF)
        # unfold level-2 rows of Q1/Q3
        q1 = np.concatenate(
            [q[:, :, 0, :Q2F] + q[:, :, 0, Q2F:],
             (q[:, :, 0, :Q2F] - q[:, :, 0, Q2F:])[:, :, ::-1]], axis=2)
        q3 = np.concatenate(
            [q[:, :, 2, :Q2F] + q[:, :, 2, Q2F:],
             (q[:, :, 2, :Q2F] - q[:, :, 2, Q2F:])[:, :, ::-1]], axis=2)
        q2, q4 = q[:, :, 1], q[:, :, 3]
        blk = out[i * per_core : (i + 1) * per_core]
        blk[:, :, :QF, :QF] = q1 + q2
        blk[:, :, QF:, :QF] = (q1 - q2)[:, :, ::-1, :]
        blk[:, :, :QF, QF:] = (q3 + q4)[:, :, :, ::-1]
        blk[:, :, QF:, QF:] = (q3 - q4)[:, :, ::-1, ::-1]
    if _trace:
        return out, res
    return out


# revision 18
# speedup vs baseline: 1.0672x; 1.0672x over previous
"""DCT-based 1.25x upsample (2D DCT-II -> zero-pad spectrum -> 2D IDCT).

The reference computation is linear per (b, c) slice: out = M @ x @ M^T with
M = E960[:, :768] @ D768 (960x768). M is *centrosymmetric*
(M[959-i, 767-n] = M[i, n]), so the symmetric/antisymmetric fold halves the
matmul FLOPs:

    MP = (M[:480, :384] + M[:480, 767:383:-1]) / 2     [480, 384]
    MM = (M[:480, :384] - M[:480, 767:383:-1]) / 2
    x_pq = (row-fold p)(col-fold q)(x)                 4 tiles of [384, 384]

    P1 = x_pp MP^T   P2 = x_pm MM^T   P3 = x_mp MP^T   P4 = x_mm MM^T
    A = P1+P2  C = P1-P2  B = P3+P4  D = P3-P4         [384, 480] each
    Q1 = MP A   Q2 = MM B   Q3 = MP C   Q4 = MM D      [480, 480] each
    out quadrants = (Q1 +- Q2, Q3 +- Q4) with flips    (done on HOST)

MP is itself centrosymmetric (MM is not), giving a second-level fold on the
Q1/Q3 products: with x_pp/x_pm row-PERMUTED on the host into level-2 folded
order (rows [0:192) = r[n]+r[383-n], [192:384) = r[n]-r[383-n]), A and C come
out of stage 1 already row-folded, and

    Q1 = unfold_i([MP2p @ A[:192] ; MP2m @ A[192:]])   MP2{p,m} [240, 192]

costs 8 matmuls instead of 12 (the unfold is part of the host assembly).

Host does all O(N^2) folds/permutes/unfolds/flips (free, like the layout
striping); the device runs bf16 matmuls (1 PE cycle/row) plus the minimal
PSUM drains: stage 1 needs one ScalarE cast + two DVE PSUM ops per step
(A/C butterfly feeds stage 2 on-device), stage 2 is pure casts split between
ScalarE and DVE. GpSimd has no PSUM port and DMA no PSUM route, so those
stay off the drain path.
"""

import numpy as np
import ml_dtypes

import concourse.bass as bass  # noqa: F401  (engine types route via nc)
import concourse.mybir as mybir
import concourse.tile as tile
from concourse import bacc
from concourse.bass_utils import run_bass_kernel_spmd

# Problem shape (hardcoded per contract)
B, C, H = 16, 3, 768
OUT = 960  # H * 1.25
N_CORES = 8
SLICES = (B * C) // N_CORES  # 6 per core

P = 128
HF = H // 2     # 384: folded input length
QF = OUT // 2   # 480: folded output length
H2 = HF // 2    # 192: doubly folded (level-2 contraction)
Q2F = QF // 2   # 240: doubly folded output rows
KT = HF // P    # 3 contraction tiles of 128
NT1 = HF // P   # 3 stage-1 output-row tiles
M2 = 120
MT2 = QF // M2  # 4 stage-2 output-row tiles

DT = mybir.dt.bfloat16
BF16 = ml_dtypes.bfloat16


def _build_consts():
    """Returns (mt, mt2a, mt2b) host arrays (bf16).

    mt   [128, 2*KT*QF]: MP^T / MM^T, K striped -> (kl, p). Used as the
         stage-1 moving operand and the stage-2 lhsT for Q2/Q4.
    mt2a [128, 2*Q2F], mt2b [64, 2*Q2F]: level-2 constants MP2p^T/MP2m^T
         [192, 240] split into the K-chunks matching A's row tiling:
         f+ contracts A rows 0:192  = [kl0 (128) ; kl1[:64]  (64)]
         f- contracts A rows 192:384 = [kl1[64:] (64) ; kl2 (128)]
         mt2a[:, 0] = MP2pT[0:128]   mt2b[:, 0] = MP2pT[128:192]
         mt2b[:, 1] = MP2mT[0:64]    mt2a[:, 1] = MP2mT[64:192]
    """
    n = np.arange(H, dtype=np.float64)
    k = np.arange(H, dtype=np.float64)[:, None]
    D = 2.0 * np.cos(np.pi * (2.0 * n[None, :] + 1.0) * k / (2.0 * H))

    n2 = np.arange(OUT, dtype=np.float64)[:, None]
    k2 = np.arange(OUT, dtype=np.float64)[None, :]
    E = np.cos(np.pi * (2.0 * n2 + 1.0) * k2 / (2.0 * OUT)) / OUT
    E[:, 0] = 1.0 / (2.0 * OUT)

    M = E[:, :H] @ D  # [960, 768]
    MP = (M[:QF, :HF] + M[:QF, H - 1 : HF - 1 : -1]) / 2.0  # [480, 384]
    MM = (M[:QF, :HF] - M[:QF, H - 1 : HF - 1 : -1]) / 2.0
    mt = np.stack([MP.T, MM.T])  # [2, 384, 480]
    mt = mt.reshape(2, KT, P, QF).transpose(2, 0, 1, 3)
    mt = np.ascontiguousarray(mt.reshape(P, 2 * KT * QF)).astype(BF16)

    MP2pT = ((MP[:Q2F, :H2] + MP[:Q2F, HF - 1 : H2 - 1 : -1]) / 2.0).T  # [192, 240]
    MP2mT = ((MP[:Q2F, :H2] - MP[:Q2F, HF - 1 : H2 - 1 : -1]) / 2.0).T
    mt2a = np.stack([MP2pT[0:128], MP2mT[64:192]], axis=1)  # [128, 2, 240]
    # 64-row chunks parked at the base partition their rhs slice uses
    # (matmul requires lhsT/rhs base partitions to match):
    # [:64, 0] = MP2pT[128:192] (rhs base 0), [64:, 1] = MP2mT[0:64] (base 64)
    mt2b = np.zeros((P, 2, Q2F), dtype=np.float64)
    mt2b[:64, 0] = MP2pT[128:192]
    mt2b[64:, 1] = MP2mT[0:64]
    mt2a = np.ascontiguousarray(mt2a.reshape(P, 2 * Q2F)).astype(BF16)
    mt2b = np.ascontiguousarray(mt2b.reshape(P, 2 * Q2F)).astype(BF16)
    return mt, mt2a, mt2b


def _fold_inputs(x: np.ndarray) -> np.ndarray:
    """Host fold + lhsT striping: [B*C, 128, 4*KT*HF] bf16.

    Product order t=0..3 pairs (x_pp, MPt), (x_pm, MMt), (x_mp, MPt),
    (x_mm, MMt). x_pp and x_pm (the A/C path) are additionally row-permuted
    into level-2 folded order. lhsT layout: x_sb[p, t, kl, n] =
    x_t[n, kl*128+p].
    """
    xr = x.reshape(B * C, H, H)
    fp = xr[:, :, :HF] + xr[:, :, H - 1 : HF - 1 : -1]  # col fold
    fm = xr[:, :, :HF] - xr[:, :, H - 1 : HF - 1 : -1]
    xpp = fp[:, :HF] + fp[:, H - 1 : HF - 1 : -1]  # row fold
    xmp = fp[:, :HF] - fp[:, H - 1 : HF - 1 : -1]
    xpm = fm[:, :HF] + fm[:, H - 1 : HF - 1 : -1]
    xmm = fm[:, :HF] - fm[:, H - 1 : HF - 1 : -1]
    # level-2 row fold (permute) on the A/C path
    xpp = np.concatenate(
        [xpp[:, :H2] + xpp[:, HF - 1 : H2 - 1 : -1],
         xpp[:, :H2] - xpp[:, HF - 1 : H2 - 1 : -1]], axis=1)
    xpm = np.concatenate(
        [xpm[:, :H2] + xpm[:, HF - 1 : H2 - 1 : -1],
         xpm[:, :H2] - xpm[:, HF - 1 : H2 - 1 : -1]], axis=1)
    xs = np.stack([xpp, xpm, xmp, xmm], axis=1)  # [B*C, 4, n, m]
    xt = xs.transpose(0, 1, 3, 2)  # lhsT: [B*C, 4, m, n]
    xt = xt.reshape(B * C, 4, KT, P, HF).transpose(0, 3, 1, 2, 4)
    return np.ascontiguousarray(xt.reshape(B * C, P, 4 * KT * HF)).astype(BF16)


def _build_program():
    nc = bacc.Bacc(None, target_bir_lowering=False, debug=False)

    x_ext = nc.dram_tensor("x", [SLICES, P, 4 * KT * HF], DT, kind="ExternalInput")
    mt_ext = nc.dram_tensor("mt", [P, 2 * KT * QF], DT, kind="ExternalInput")
    mt2a_ext = nc.dram_tensor("mt2a", [P, 2 * Q2F], DT, kind="ExternalInput")
    mt2b_ext = nc.dram_tensor("mt2b", [P, 2 * Q2F], DT, kind="ExternalInput")
    out_ext = nc.dram_tensor("out", [SLICES, 4, QF, QF], DT, kind="ExternalOutput")

    with tile.TileContext(nc) as tc:
        with (
            tc.tile_pool(name="const", bufs=1) as const_pool,
            tc.tile_pool(name="xp", bufs=3) as x_pool,
            tc.tile_pool(name="rp", bufs=2) as r_pool,
            tc.tile_pool(name="op", bufs=10) as o_pool,
            tc.tile_pool(name="ps", bufs=8, space="PSUM") as psum_pool,
        ):
            x_dram = x_ext[:].rearrange("s p (t k n) -> s p t k n", t=4, k=KT)

            # Slice-0 x rides two HWDGE queues so stage-1 pair 0's half
            # lands by ~9us; constants share the scalar queue ahead of the
            # second half. Later slices prefetch on sync with plenty of slack.
            x_first = x_pool.tile([P, 4, KT, HF], DT, tag="x")
            nc.sync.dma_start(x_first[:, 0:2], x_dram[0][:, 0:2])
            nc.sync.dma_start(x_first[:, 2:4], x_dram[0][:, 2:4])

            mt_dram = mt_ext[:].rearrange("p (c k j) -> p c k j", c=2, k=KT)
            mt_sb = const_pool.tile([P, 2, KT, QF], DT, name="mt")
            nc.scalar.dma_start(mt_sb[:], mt_dram[:])
            mt2a_sb = const_pool.tile([P, 2, Q2F], DT, name="mt2a")
            nc.scalar.dma_start(mt2a_sb[:], mt2a_ext[:].rearrange("p (c j) -> p c j", c=2))
            mt2b_sb = const_pool.tile([P, 2, Q2F], DT, name="mt2b")
            nc.scalar.dma_start(mt2b_sb[:], mt2b_ext[:].rearrange("p (c j) -> p c j", c=2))


            # PE warmup: dummy matmuls keep the tensor engine busy while the
            # first loads land, so the HAM clock gate is already at 2.4 GHz.
            warm_m = const_pool.tile([P, QF], DT, name="warm_m")
            nc.vector.memset(warm_m[:], 0.0)
            warm_ps = psum_pool.tile([P, QF], mybir.dt.float32, tag="ps", name="warm_ps")
            for _ in range(14):
                nc.tensor.matmul(warm_ps[:], warm_m[:, :P], warm_m[:], start=True, stop=True)

            for s in range(SLICES):
                if s == 0:
                    x_sb = x_first
                else:
                    x_sb = x_pool.tile([P, 4, KT, HF], DT, tag="x")
                    nc.sync.dma_start(x_sb[:], x_dram[s])

                # Stage 1: P_t = x_t @ {MP,MM}^T, combined in pairs into
                # r_sb[:, u]: u=0:A, u=1:C (level-2-row-folded), u=2:B, u=3:D,
                # K-striped for stage 2: r_sb[p, u, nt, j] = U[nt*128+p, j].
                r_sb = r_pool.tile([P, 4, KT, QF], DT, tag="r")
                for pair in range(2):
                    for nt in range(NT1):
                        ps_a = psum_pool.tile([P, QF], mybir.dt.float32, tag="ps")
                        ps_b = psum_pool.tile([P, QF], mybir.dt.float32, tag="ps")
                        for kl in range(KT):
                            nc.tensor.matmul(
                                ps_a[:],
                                x_sb[:, 2 * pair, kl, nt * P : (nt + 1) * P],
                                mt_sb[:, 0, kl, :],
                                start=(kl == 0),
                                stop=(kl == KT - 1),
                            )
                        for kl in range(KT):
                            nc.tensor.matmul(
                                ps_b[:],
                                x_sb[:, 2 * pair + 1, kl, nt * P : (nt + 1) * P],
                                mt_sb[:, 1, kl, :],
                                start=(kl == 0),
                                stop=(kl == KT - 1),
                            )
                        # Minimal drain: ScalarE casts ps_b (DVE has a single
                        # PSUM read port, dual-PSUM tensor_tensor is illegal);
                        # DVE butterflies PSUM + SBUF -> SBUF bf16.
                        pb_sb = o_pool.tile([P, QF], mybir.dt.float32, tag="o")
                        nc.scalar.copy(pb_sb[:], ps_b[:])
                        nc.vector.tensor_add(
                            out=r_sb[:, 2 * pair, nt, :], in0=ps_a[:], in1=pb_sb[:]
                        )
                        nc.vector.tensor_sub(
                            out=r_sb[:, 2 * pair + 1, nt, :], in0=ps_a[:], in1=pb_sb[:]
                        )

                # Stage 2: Q1 = MP A, Q3 = MP C via level-2 fold (8 matmuls,
                # output rows = [f+ (240) ; f- (240)], host unfolds);
                # Q2 = MM B, Q4 = MM D natural (12 matmuls). Drains are pure
                # casts, alternating ScalarE / DVE. Host does the +- butterfly.
                for qi, u, folded in ((0, 0, True), (1, 2, False), (2, 1, True), (3, 3, False)):
                    for mi in range(MT2):
                        ps = psum_pool.tile([P, QF], mybir.dt.float32, tag="ps")
                        po = ps[:M2, :]
                        if folded:
                            half, mi2b = divmod(mi, 2)
                            msl = slice(mi2b * M2, (mi2b + 1) * M2)
                            if half == 0:  # f+ : A rows 0:192
                                nc.tensor.matmul(
                                    po, mt2a_sb[:, 0, msl],
                                    r_sb[:, u, 0, :], start=True, stop=False,
                                )
                                nc.tensor.matmul(
                                    po, mt2b_sb[:64, 0, msl],
                                    r_sb[:64, u, 1, :], start=False, stop=True,
                                )
                            else:  # f- : A rows 192:384
                                nc.tensor.matmul(
                                    po, mt2b_sb[64:, 1, msl],
                                    r_sb[64:, u, 1, :], start=True, stop=False,
                                )
                                nc.tensor.matmul(
                                    po, mt2a_sb[:, 1, msl],
                                    r_sb[:, u, 2, :], start=False, stop=True,
                                )
                        else:
                            for kl in range(KT):
                                nc.tensor.matmul(
                                    po,
                                    mt_sb[:, 1, kl, mi * M2 : (mi + 1) * M2],
                                    r_sb[:, u, kl, :],
                                    start=(kl == 0),
                                    stop=(kl == KT - 1),
                                )
                        oc = o_pool.tile([M2, QF], DT, tag="o")
                        if (qi * MT2 + mi) % 2 == 0:
                            nc.scalar.copy(oc[:], po)
                            nc.scalar.dma_start(
                                out_ext[s, qi, mi * M2 : (mi + 1) * M2, :], oc[:]
                            )
                        else:
                            nc.vector.tensor_copy(oc[:], po)
                            nc.sync.dma_start(
                                out_ext[s, qi, mi * M2 : (mi + 1) * M2, :], oc[:]
                            )

    nc.compile()
    return nc


_CACHE: dict = {}


def _get_program():
    if "nc" not in _CACHE:
        _CACHE["nc"] = _build_program()
        _CACHE["consts"] = _build_consts()
    return _CACHE["nc"], _CACHE["consts"]


def kernel(x: np.ndarray, _trace: bool = False):
    assert x.shape == (B, C, H, H), x.shape
    nc, (mt, mt2a, mt2b) = _get_program()
    x = np.ascontiguousarray(x, dtype=np.float32)
    x_arr = _fold_inputs(x)
    per_core = B // N_CORES
    in_maps = [
        {"x": x_arr[i * SLICES : (i + 1) * SLICES], "mt": mt, "mt2a": mt2a,
         "mt2b": mt2b}
        for i in range(N_CORES)
    ]
    res = run_bass_kernel_spmd(nc, in_maps, list(range(N_CORES)), trace=_trace)
    out = np.empty((B, C, OUT, OUT), dtype=np.float32)
    for i in range(N_CORES):
        q = np.asarray(res.results[i]["out"]).astype(np.float32)
        q = q.reshape(per_core, C, 4, QF, QF)
        # unfold level-2 rows of Q1/Q3
        q1 = np.concatenate(
            [q[:, :, 0, :Q2F] + q[:, :, 0, Q2F:],
             (q[:, :, 0, :Q2F] - q[:, :, 0, Q2F:])[:, :, ::-1]], axis=2)
        q3 = np.concatenate(
            [q[:, :, 2, :Q2F] + q[:, :, 2, Q2F:],
             (q[:, :, 2, :Q2F] - q[:, :, 2, Q2F:])[:, :, ::-1]], axis=2)
        q2, q4 = q[:, :, 1], q[:, :, 3]
        blk = out[i * per_core : (i + 1) * per_core]
        blk[:, :, :QF, :QF] = q1 + q2
        blk[:, :, QF:, :QF] = (q1 - q2)[:, :, ::-1, :]
        blk[:, :, :QF, QF:] = (q3 + q4)[:, :, :, ::-1]
        blk[:, :, QF:, QF:] = (q3 - q4)[:, :, ::-1, ::-1]
    if _trace:
        return out, res
    return out
